# revision 14
# baseline (speedup 1.0000x reference)
"""Trainium2 Bass kernel: transformer block with dilated (parity-strided,
banded, causal) attention.

v2 design notes (vs the earlier revision):
  * Both LayerNorms are folded into the surrounding GEMMs WITHOUT the
    mean-subtraction pass: each projection PSUM chain gets one extra rank-1
    accumulation matmul  out += mu ⊗ (-rowsum(W))  so the GEMMs consume the
    plain bf16 cast of x and never wait on the LN statistics (the rstd scale
    lands in the epilogue, the mu term is the chain tail).  This removes the
    stats->sub->GEMM serial wall that left the PE idle (and HAM-throttled to
    1.2 GHz) for ~30us per LN.
  * The dilated/causal mask is applied inside the score accumulation as an
    additive -49152 matmul (identity lhsT x host-built additive mask), so the
    gpsimd multiply (24 x 1.5us, plus a cross-engine hop in the critical
    chain) is gone; exp() of a masked score underflows to exactly 0.
  * rstd = exp(-0.5*ln(var+eps)) so the only ACT table sets used are
    natural_log_exp_and_others (stats + attention exp) and gelu_and_others
    (FFN); both are prefetched by dummy activations to hide the ~2.7us
    table-set DMA.
  * LN statistics use one PSUM bank with four concurrent M=1 column-tiled
    accumulation chains (sum/sumsq x two token chunks at partitions
    0/32/64/96).
  * Softmax denominators are collected per head into r_all and broadcast
    across partitions with one 2-row indicator matmul per feature tile
    (6 matmuls instead of 12).
"""

import numpy as np
import ml_dtypes

import concourse.bass as bass
import concourse.bacc as bacc
import concourse.mybir as mybir
import concourse.tile as tile
from concourse.bass_utils import run_bass_kernel_spmd

BF16NP = ml_dtypes.bfloat16
F32 = mybir.dt.float32
BF16 = mybir.dt.bfloat16
AF = mybir.ActivationFunctionType
OP = mybir.AluOpType

P = 128
B, L, E = 2, 2048, 768
ET = E // P            # 6 tiles over E
H, D = 12, 64
MLP = 4 * E            # 3072
MT = MLP // P          # 24
OWN = 512              # tokens owned per core
HALO = 256             # preceding-context tokens
SLAB = OWN + HALO      # 768
EPS = 1e-5
N_CORES = 8
MASKNEG = -49152.0     # exact in bf16; exp((s+MASKNEG)/8) == 0


def _fold2(apv):
    """[.., T] -> [.., 2, T//2] parity view of a stride-1 token axis."""
    return apv.rearrange("... (t two) -> ... two t", two=2)


def build_program():
    nc = bacc.Bacc("TRN2", target_bir_lowering=False, debug=False)

    xT = nc.dram_tensor("xT", [E, SLAB], F32, kind="ExternalInput").ap()
    qkv_wT = nc.dram_tensor("qkv_wT", [E, 3 * E], BF16, kind="ExternalInput").ap()
    out_wT = nc.dram_tensor("out_wT", [E, E], BF16, kind="ExternalInput").ap()
    ffn_w1T = nc.dram_tensor("ffn_w1T", [E, MLP], BF16, kind="ExternalInput").ap()
    ffn_w2T = nc.dram_tensor("ffn_w2T", [MLP, E], BF16, kind="ExternalInput").ap()
    qkv_b = nc.dram_tensor("qkv_b", [3 * E], F32, kind="ExternalInput").ap()
    out_b = nc.dram_tensor("out_b", [E], F32, kind="ExternalInput").ap()
    ffn_b1 = nc.dram_tensor("ffn_b1", [MLP], F32, kind="ExternalInput").ap()
    ffn_b2 = nc.dram_tensor("ffn_b2", [E], F32, kind="ExternalInput").ap()
    maskT = nc.dram_tensor("maskT", [2, 2, P, P], BF16, kind="ExternalInput").ap()
    identT = nc.dram_tensor("identT", [P, P], BF16, kind="ExternalInput").ap()
    ind2T = nc.dram_tensor("ind2T", [65, P], F32, kind="ExternalInput").ap()
    negsT = nc.dram_tensor("negsT", [1, 3 * E], BF16, kind="ExternalInput").ap()
    negs1T = nc.dram_tensor("negs1T", [1, MLP], BF16, kind="ExternalInput").ap()
    yT = nc.dram_tensor("yT", [E, OWN], F32, kind="ExternalOutput").ap()

    with tile.TileContext(nc) as tc:
        _emit(tc, xT, qkv_wT, out_wT, ffn_w1T, ffn_w2T,
              qkv_b, out_b, ffn_b1, ffn_b2, maskT, identT, ind2T,
              negsT, negs1T, yT)
    nc.compile()
    return nc


def _emit(tc, xT, qkv_wT, out_wT, ffn_w1T, ffn_w2T,
          qkv_b, out_b, ffn_b1, ffn_b2, maskT, identT, ind2T,
          negsT, negs1T, yT):
    from contextlib import ExitStack
    ctx = ExitStack()
    nc = tc.nc

    sing = ctx.enter_context(tc.tile_pool(name="sing", bufs=1))
    xw2_pool = ctx.enter_context(tc.tile_pool(name="xw2", bufs=1))
    wq_pool = ctx.enter_context(tc.tile_pool(name="wq", bufs=3))
    wv_pool = ctx.enter_context(tc.tile_pool(name="wv", bufs=1))
    w1_pool = ctx.enter_context(tc.tile_pool(name="w1", bufs=3))
    sq_pool = ctx.enter_context(tc.tile_pool(name="sq", bufs=2))
    ex_pool = ctx.enter_context(tc.tile_pool(name="ex", bufs=3))
    row_pool = ctx.enter_context(tc.tile_pool(name="rows", bufs=3))
    ow_pool = ctx.enter_context(tc.tile_pool(name="owp", bufs=2))
    ft_pool = ctx.enter_context(tc.tile_pool(name="ftmp", bufs=3))
    den_pool = ctx.enter_context(tc.tile_pool(name="den", bufs=3))

    # PSUM: 8 banks total = g:3 + st:1 + sc:2 + pv:2
    ps_main = ctx.enter_context(tc.tile_pool(name="psg", bufs=3, space="PSUM"))
    ps_st = ctx.enter_context(tc.tile_pool(name="psst", bufs=1, space="PSUM"))
    ps_attn = ctx.enter_context(tc.tile_pool(name="pssc", bufs=2, space="PSUM"))
    ps_pv = ctx.enter_context(tc.tile_pool(name="pspv", bufs=2, space="PSUM"))

    # ---------------- phase 0: input DMAs + constants ----------------
    x_sb = xw2_pool.tile([P, ET, SLAB], F32, tag="xw2", name="x_sb")
    xT_v = xT.rearrange("(o p) t -> p o t", p=P)
    for et in range(ET):
        nc.sync.dma_start(out=x_sb[:, et, :], in_=xT_v[:, et, :])

    qkvb_sb = sing.tile([P, 18], F32, tag="qkvb")
    nc.sync.dma_start(out=qkvb_sb, in_=qkv_b.rearrange("(o p) -> p o", p=P))
    outb_sb = sing.tile([P, ET], F32, tag="outb")
    nc.sync.dma_start(out=outb_sb, in_=out_b.rearrange("(o p) -> p o", p=P))
    b1_sb = sing.tile([P, MT], F32, tag="b1")
    nc.sync.dma_start(out=b1_sb, in_=ffn_b1.rearrange("(o p) -> p o", p=P))
    b2_sb = sing.tile([P, ET], F32, tag="b2")
    nc.sync.dma_start(out=b2_sb, in_=ffn_b2.rearrange("(o p) -> p o", p=P))

    ident_sb = sing.tile([P, P], BF16, tag="ident")
    nc.sync.dma_start(out=ident_sb, in_=identT)
    ind2_sb = sing.tile([65, P], F32, tag="ind2")
    nc.sync.dma_start(out=ind2_sb, in_=ind2T)
    negs_sb = sing.tile([1, 3 * ET, P], BF16, tag="negs")
    nc.sync.dma_start(out=negs_sb.rearrange("a b c -> a (b c)"), in_=negsT)
    negs1_sb = sing.tile([1, MT, P], BF16, tag="negs1")
    nc.sync.dma_start(out=negs1_sb.rearrange("a b c -> a (b c)"), in_=negs1T)

    # additive masks replicated over the head-pair dim: [key, qb, h2, kb, q]
    masks_sb = sing.tile([P, 2, 2, 2, P], BF16, tag="masks")
    for qb in range(2):
        for hrep in range(2):
            for kb in range(2):
                nc.sync.dma_start(out=masks_sb[:, qb, hrep, kb, :],
                                  in_=maskT[qb, kb])

    ones_pf = sing.tile([P, 1], BF16, tag="ones_pf")
    nc.vector.memset(ones_pf, 1.0)
    ones_row = sing.tile([1, P], BF16, tag="ones_row")
    nc.vector.memset(ones_row, 1.0)
    eps_sb = sing.tile([1, 1], F32, tag="eps")
    nc.vector.memset(eps_sb, EPS)
    scr_sb = sing.tile([1, 2], F32, tag="scr")

    # preload the natural_log_exp table set while input DMAs stream
    nc.scalar.activation(scr_sb[:, 0:1], eps_sb, AF.Exp)

    # dummy matmuls HAM-warm the PE clock while the input DMAs stream
    warm_src = sing.tile([P, 256], BF16, tag="warm_src")
    nc.gpsimd.memset(warm_src, 0.0)
    const_bf = nc.const_aps.aps[(mybir.dt.bfloat16, 1.0)]
    wps = ps_main.tile([P, 512], F32, tag="g", name="warm_ps")
    for wi in range(10):
        nc.tensor.matmul(wps[0:1, 0:256], const_bf, warm_src,
                         start=True, stop=True)

    # ---------------- folded layernorm statistics ----------------
    # Four concurrent M=1 accumulation chains into one PSUM bank, column
    # groups 0/32/64/96: sum(x) chunk0, sum(x^2) chunk0, sum(x) chunk1,
    # sum(x^2) chunk1.  Chunks: [(0,512)] for 512 tokens, +[ (512,256) ]
    # for the 768-token slab.
    def emit_ln_stats_mm(xbf, src, ntok, name):
        chunks = [(0, 512)] + ([(512, ntok - 512)] if ntok > 512 else [])
        st = ps_st.tile([P, 512], F32, tag="st", name=name)
        for et in range(ET):
            nc.vector.tensor_copy(out=xbf[:, et, :ntok], in_=src[:, et, :ntok])
            xsq = sq_pool.tile([P, ntok], BF16, tag="sq")
            nc.scalar.activation(xsq, src[:, et, :ntok], AF.Square)
            for ci, (c0, cl) in enumerate(chunks):
                r0 = 64 * ci
                nc.tensor.matmul(st[r0:r0 + 1, :cl], ones_pf,
                                 xbf[:, et, c0:c0 + cl],
                                 start=(et == 0), stop=(et == ET - 1),
                                 tile_position=(0, r0))
                nc.tensor.matmul(st[r0 + 32:r0 + 33, :cl], ones_pf,
                                 xsq[:, c0:c0 + cl],
                                 start=(et == 0), stop=(et == ET - 1),
                                 tile_position=(0, r0 + 32))
        return st, chunks

    def emit_ln_epilogue(st, chunks, ninv, mu_row, arep_sb):
        """mu_row [1,ntok] bf16; arep_sb [P,ntok] bf16 (rstd broadcast)."""
        for ci, (c0, cl) in enumerate(chunks):
            r0 = 64 * ci
            nc.scalar.activation(mu_row[:, c0:c0 + cl], st[r0:r0 + 1, :cl],
                                 AF.Copy, scale=ninv)
            musq = row_pool.tile([1, 512], F32, tag="row")
            nc.scalar.activation(musq[:, :cl], st[r0:r0 + 1, :cl], AF.Square,
                                 scale=ninv)
            var = row_pool.tile([1, 512], F32, tag="row")
            nc.vector.scalar_tensor_tensor(
                out=var[:, :cl], in0=st[r0 + 32:r0 + 33, :cl], scalar=ninv,
                in1=musq[:, :cl], op0=OP.mult, op1=OP.subtract)
            lnv = row_pool.tile([1, 512], F32, tag="row")
            nc.scalar.activation(lnv[:, :cl], var[:, :cl], AF.Ln, bias=eps_sb)
            af = row_pool.tile([1, 512], F32, tag="row")
            nc.scalar.activation(af[:, :cl], lnv[:, :cl], AF.Exp, scale=-0.5)
            a = row_pool.tile([1, 512], BF16, tag="rowb")
            nc.vector.tensor_copy(out=a[:, :cl], in_=af[:, :cl])
            arep = ps_main.tile([P, 512], F32, tag="g")
            nc.tensor.matmul(arep[:, :cl], ones_row, a[:, :cl],
                             start=True, stop=True)
            nc.vector.tensor_copy(out=arep_sb[:, c0:c0 + cl],
                                  in_=arep[:, :cl])
        return a  # last rstd row (bf16) for table-prefetch ordering

    # ---------------- phase 1: LN1 stats ----------------
    x1_bf = sing.tile([P, ET, SLAB], BF16, tag="x1_bf")
    mu1 = sing.tile([1, SLAB], BF16, tag="mu1")
    a1rep = sing.tile([P, SLAB], BF16, tag="a1rep")
    st1, chunks1 = emit_ln_stats_mm(x1_bf, x_sb, SLAB, "st1")
    emit_ln_epilogue(st1, chunks1, 1.0 / E, mu1, a1rep)

    # rstd as per-partition columns for the V epilogue (tokens on partitions)
    acol_ps = ps_main.tile([P, 512], F32, tag="g", name="acol_ps")
    one1 = ones_row[:, 0:1]
    for par in range(2):
        for kb in range(3):
            nc.tensor.matmul(acol_ps[:, par * 3 + kb:par * 3 + kb + 1],
                             _fold2(a1rep[0:1, :])[:, par, kb * P:(kb + 1) * P],
                             one1, start=(par == 0 and kb == 0),
                             stop=(par == 1 and kb == 2))
    acol_sb = sing.tile([P, 2, 3], F32, tag="acol")
    nc.vector.tensor_copy(out=acol_sb.rearrange("p a b -> p (a b)"),
                          in_=acol_ps[:, 0:6])

    # ---------------- phase 2: QKV projections (LN folded in) ----------------
    wq_view = qkv_wT.rearrange("(o p) f -> p o f", p=P)

    k_sb = sing.tile([P, ET, SLAB], BF16, tag="k_sb")
    for ft in range(ET):
        wt = wq_pool.tile([P, ET, P], BF16, tag="wq")
        nc.sync.dma_start(out=wt, in_=wq_view[:, :, E + ft * P:E + (ft + 1) * P])
        for c0, cl in [(0, 512), (512, 256)]:
            ps = ps_main.tile([P, 512], F32, tag="g")
            for et in range(ET):
                nc.tensor.matmul(ps[:, :cl], wt[:, et, :],
                                 x1_bf[:, et, c0:c0 + cl],
                                 start=(et == 0), stop=False)
            nc.tensor.matmul(ps[:, :cl], negs_sb[:, ET + ft, :],
                             mu1[:, c0:c0 + cl], start=False, stop=True)
            t = ft_pool.tile([P, 512], BF16, tag="ftb")
            nc.vector.tensor_mul(t[:, :cl], ps[:, :cl], a1rep[:, c0:c0 + cl])
            nc.scalar.activation(k_sb[:, ft, c0:c0 + cl], t[:, :cl],
                                 AF.Identity, bias=qkvb_sb[:, 6 + ft:7 + ft])

    q_sb = sing.tile([P, ET, OWN], BF16, tag="q_sb")
    for ft in range(ET):
        wt = wq_pool.tile([P, ET, P], BF16, tag="wq")
        nc.sync.dma_start(out=wt, in_=wq_view[:, :, ft * P:(ft + 1) * P])
        ps = ps_main.tile([P, 512], F32, tag="g")
        for et in range(ET):
            nc.tensor.matmul(ps, wt[:, et, :], x1_bf[:, et, HALO:SLAB],
                             start=(et == 0), stop=False)
        nc.tensor.matmul(ps, negs_sb[:, ft, :], mu1[:, HALO:SLAB],
                         start=False, stop=True)
        t = ft_pool.tile([P, 512], BF16, tag="ftb")
        nc.vector.tensor_mul(t, ps, a1rep[:, HALO:SLAB])
        nc.scalar.activation(q_sb[:, ft, :], t, AF.Identity,
                             bias=qkvb_sb[:, ft:ft + 1])

    # V in [token, feature] orientation; LN fold: rank-1 mu term via folded
    # mu as lhsT, rstd applies per-partition via ACT scale.  V bias is folded
    # into out_b on the host.
    v_sb = sing.tile([P, 2, 3, H, D + 1], BF16, tag="v_sb")
    nc.vector.memset(v_sb[:, :, :, :, D:D + 1], 1.0)
    for vc0, vcl in [(0, 512), (512, 256)]:
        wt = wv_pool.tile([P, ET, vcl], BF16, tag=f"wv{vcl}", name=f"wtv{vcl}")
        nc.sync.dma_start(out=wt,
                          in_=wq_view[:, :, 2 * E + vc0:2 * E + vc0 + vcl])
        for kb in range(3):
            for par in range(2):
                ps = ps_main.tile([P, 512], F32, tag="g")
                for et in range(ET):
                    hblk = _fold2(x1_bf[:, et, :])[:, par, kb * P:(kb + 1) * P]
                    nc.tensor.matmul(ps[:, :vcl], hblk, wt[:, et, :vcl],
                                     start=(et == 0), stop=False)
                nsv = negs_sb[:, 2 * ET + vc0 // P:2 * ET + (vc0 + vcl) // P,
                              :].rearrange("a b c -> a (b c)")
                nc.tensor.matmul(ps[:, :vcl],
                                 _fold2(mu1)[:, par, kb * P:(kb + 1) * P],
                                 nsv, start=False, stop=True)
                nc.scalar.activation(
                    v_sb[:, par, kb, vc0 // D:(vc0 + vcl) // D, 0:D],
                    ps[:, :vcl].rearrange("p (h d) -> p h d", d=D), AF.Copy,
                    scale=acol_sb[:, par, kb:kb + 1])

    outw_v = out_wT.rearrange("(o p) e -> p o e", p=P)

    # ---------------- phase 3: dilated attention ----------------
    o_sb = sing.tile([P, ET, OWN], BF16, tag="o_sb")
    r_all = sing.tile([65, ET, 2, 256], F32, tag="r_all")
    pairs = [(0, 2), (1, 3), (4, 6), (5, 7), (8, 10), (9, 11)]
    for pi, (h0, h1) in enumerate(pairs):
        kt = h0 // 2
        ro = D * (h0 % 2)
        slot = h0 % 2
        for par in range(2):
            for qb in range(2):
                sc = ps_attn.tile([P, 2, 2, P], F32, tag="sc")
                nc.tensor.matmul(sc.rearrange("p a b c -> p (a b c)"),
                                 ident_sb, masks_sb[:, qb].rearrange(
                                     "p a b c -> p (a b c)"),
                                 start=True, stop=False)
                for hi, h in enumerate((h0, h1)):
                    ktt = h // 2
                    qv = _fold2(q_sb[ro:ro + D, ktt, :])[:, par,
                                                         qb * P:(qb + 1) * P]
                    kv = _fold2(k_sb[ro:ro + D, ktt, :])
                    for kbi, kb in enumerate((qb, qb + 1)):
                        nc.tensor.matmul(
                            sc[:, hi, kbi, :],
                            kv[:, par, kb * P:(kb + 1) * P], qv,
                            start=False,
                            stop=(hi == 1 and kbi == 1))
                ex = ex_pool.tile([P, 2, 2, P], BF16, tag="ex")
                nc.scalar.activation(ex, sc, AF.Exp, scale=1.0 / np.sqrt(D))
                pv = ps_pv.tile([D + 1, 2, P], F32, tag="pv")
                for hi, h in enumerate((h0, h1)):
                    for kbi, kb in enumerate((qb, qb + 1)):
                        nc.tensor.matmul(
                            pv[:, hi, :], v_sb[:, par, kb, h, :],
                            ex[:, hi, kbi, :],
                            start=(hi == 0 and kbi == 0),
                            stop=(hi == 1 and kbi == 1))
                den = den_pool.tile([1, 2, P], F32, tag="den")
                nc.vector.tensor_copy(out=den, in_=pv[D:D + 1, :, :])
                if slot == 0:
                    nc.vector.reciprocal_approx_fast(
                        out=r_all[0:1, kt:kt + 2, par, qb * P:(qb + 1) * P],
                        in_=den)
                else:
                    den2 = den_pool.tile([1, 2, P], F32, tag="den2")
                    nc.vector.reciprocal_approx_fast(out=den2, in_=den)
                    nc.vector.tensor_copy(
                        out=r_all[64:65, kt:kt + 2, par, qb * P:(qb + 1) * P],
                        in_=den2)
                dst = _fold2(o_sb[ro:ro + D, kt:kt + 2, :])[:, :, par,
                                                            qb * P:(qb + 1) * P]
                nc.vector.tensor_copy(out=dst, in_=pv[0:D])
        if pi % 2 == 1:
            # both row-halves of feature tiles kt, kt+1 are complete
            for tt in (kt, kt + 1):
                rrep = ps_main.tile([P, 512], F32, tag="g")
                rrv = rrep.rearrange("m (q p) -> m q p", p=2)
                for s in range(2):
                    nc.tensor.matmul(
                        rrv, ind2_sb[64 * s:64 * s + 1, :],
                        r_all[64 * s:64 * s + 1, tt].rearrange(
                            "a p q -> a q p"),
                        start=(s == 0), stop=(s == 1))
                nc.vector.tensor_mul(o_sb[:, tt, :], o_sb[:, tt, :], rrep)

    # ---------------- phase 4: out-proj + residual ----------------
    y1_sb = sing.tile([P, ET, OWN], F32, tag="y1_sb")
    for et in range(ET):
        owt = ow_pool.tile([P, ET, P], BF16, tag="ow")
        nc.sync.dma_start(out=owt, in_=outw_v[:, :, et * P:(et + 1) * P])
        ps = ps_main.tile([P, 512], F32, tag="g")
        for ftl in range(ET):
            nc.tensor.matmul(ps, owt[:, ftl, :],
                             o_sb[:, ftl, :],
                             start=(ftl == 0), stop=(ftl == ET - 1))
        t = ft_pool.tile([P, 512], F32, tag="ft")
        nc.scalar.activation(t, ps, AF.Identity, bias=outb_sb[:, et:et + 1])
        nc.vector.tensor_add(y1_sb[:, et, :], t, x_sb[:, et, HALO:SLAB])

    # ---------------- phase 5: LN2 stats ----------------
    y1_bf = sing.tile([P, ET, OWN], BF16, tag="y1_bf")
    mu2 = sing.tile([1, OWN], BF16, tag="mu2")
    a2rep = sing.tile([P, OWN], BF16, tag="a2rep")
    st2, chunks2 = emit_ln_stats_mm(y1_bf, y1_sb, OWN, "st2")
    a2row = emit_ln_epilogue(st2, chunks2, 1.0 / E, mu2, a2rep)

    # prefetch the gelu table set now that the last exp (rstd2) is emitted
    nc.scalar.activation(scr_sb[:, 1:2], a2row[:, 0:1], AF.Gelu)

    # ---------------- phase 6: FFN1 + GELU (LN folded in) ----------------
    w2_sb = xw2_pool.tile([P, MT, E], BF16, tag="xw2", name="w2_sb")
    w2_v = ffn_w2T.rearrange("(o p) e -> p o e", p=P)
    for ktl in range(MT):
        nc.sync.dma_start(out=w2_sb[:, ktl, :], in_=w2_v[:, ktl, :])

    w1_view = ffn_w1T.rearrange("(o p) f -> p o f", p=P)
    ffnh = sing.tile([P, MT, OWN], BF16, tag="ffnh")
    for mt in range(MT):
        wt = w1_pool.tile([P, ET, P], BF16, tag="w1")
        nc.sync.dma_start(out=wt, in_=w1_view[:, :, mt * P:(mt + 1) * P])
        ps = ps_main.tile([P, 512], F32, tag="g")
        for et in range(ET):
            nc.tensor.matmul(ps, wt[:, et, :], y1_bf[:, et, :],
                             start=(et == 0), stop=False)
        nc.tensor.matmul(ps, negs1_sb[:, mt, :], mu2,
                         start=False, stop=True)
        t = ft_pool.tile([P, 512], BF16, tag="ftb")
        nc.vector.tensor_mul(t, ps, a2rep)
        nc.scalar.activation(ffnh[:, mt, :], t, AF.Gelu,
                             bias=b1_sb[:, mt:mt + 1])

    # ---------------- phase 7: FFN2 + residual + store ----------------
    yT_view = yT.rearrange("(o p) t -> p o t", p=P)
    for et in range(ET):
        ps = ps_main.tile([P, 512], F32, tag="g")
        for ktl in range(MT):
            nc.tensor.matmul(ps, w2_sb[:, ktl, et * P:(et + 1) * P],
                             ffnh[:, ktl, :],
                             start=(ktl == 0), stop=(ktl == MT - 1))
        t = ft_pool.tile([P, 512], F32, tag="ft")
        nc.scalar.activation(t, ps, AF.Identity, bias=b2_sb[:, et:et + 1])
        nc.vector.tensor_add(y1_sb[:, et, :], t, y1_sb[:, et, :])
        nc.sync.dma_start(out=yT_view[:, et, :], in_=y1_sb[:, et, :])

    ctx.close()


# ======================= host side =======================

def prep_inputs(x, ln1_w, ln1_b, qkv_w, qkv_b, out_w, out_b,
                ln2_w, ln2_b, ffn_w1, ffn_b1, ffn_w2, ffn_b2):
    """Shard/fold/cast the full inputs into 8 per-core input maps."""
    x = np.asarray(x, np.float32)
    f8 = lambda v: np.asarray(v, np.float64)

    qkv_wp = f8(qkv_w) * f8(ln1_w)[None, :]
    qkv_wT = qkv_wp.T.astype(BF16NP).copy()
    qkv_b_eff = (f8(qkv_b) + f8(qkv_w) @ f8(ln1_b)).astype(np.float32)
    out_wT = f8(out_w).T.astype(BF16NP).copy()
    out_b_eff = (f8(out_b) + f8(out_w) @ f8(qkv_b)[2 * E:]).astype(np.float32)
    ffn_w1p = f8(ffn_w1) * f8(ln2_w)[None, :]
    ffn_w1T = ffn_w1p.T.astype(BF16NP).copy()
    ffn_b1_eff = (f8(ffn_b1) + f8(ffn_w1) @ f8(ln2_b)).astype(np.float32)
    ffn_w2T = f8(ffn_w2).T.astype(BF16NP).copy()
    ffn_b2_f = np.asarray(ffn_b2, np.float32)

    # row-sums of the (ln-folded) weights, negated, for the rank-1 mu fold
    negs = (-qkv_wp.sum(axis=1)).astype(BF16NP).reshape(1, 3 * E)
    negs1 = (-ffn_w1p.sum(axis=1)).astype(BF16NP).reshape(1, MLP)

    ident = np.eye(P, dtype=BF16NP)
    ind2 = np.zeros((65, P), dtype=np.float32)
    ind2[0, 0:D] = 1.0
    ind2[64, D:P] = 1.0

    cidx = np.arange(P)[:, None]   # key (folded, within block)
    ridx = np.arange(P)[None, :]   # query (folded, within block)
    m_prev = np.where(cidx >= ridx, 0.0, MASKNEG).astype(BF16NP)
    m_diag = np.where(cidx <= ridx, 0.0, MASKNEG).astype(BF16NP)
    m_none = np.full((P, P), MASKNEG, BF16NP)

    in_maps = []
    for c in range(N_CORES):
        b, ch = divmod(c, 4)
        lo = OWN * ch - HALO
        if ch == 0:
            slab = np.concatenate(
                [np.zeros((HALO, E), np.float32), x[b, 0:OWN]], axis=0)
        else:
            slab = x[b, lo:lo + SLAB]
        xTc = np.ascontiguousarray(slab.T)

        mask = np.stack([
            np.stack([m_none if ch == 0 else m_prev, m_diag]),  # qb = 0
            np.stack([m_prev, m_diag]),                         # qb = 1
        ]).astype(BF16NP)

        in_maps.append({
            "xT": xTc,
            "qkv_wT": qkv_wT, "out_wT": out_wT,
            "ffn_w1T": ffn_w1T, "ffn_w2T": ffn_w2T,
            "qkv_b": qkv_b_eff, "out_b": out_b_eff,
            "ffn_b1": ffn_b1_eff, "ffn_b2": ffn_b2_f,
            "maskT": np.ascontiguousarray(mask),
            "identT": ident, "ind2T": ind2,
            "negsT": negs, "negs1T": negs1,
        })
    return in_maps


def gather_output(results):
    y = np.empty((B, L, E), np.float32)
    for c in range(N_CORES):
        b, ch = divmod(c, 4)
        y[b, OWN * ch:OWN * (ch + 1)] = results[c]["yT"].T
    return y


_NC_CACHE = None


def _get_program():
    global _NC_CACHE
    if _NC_CACHE is None:
        _NC_CACHE = build_program()
    return _NC_CACHE


def kernel(**inputs):
    nc = _get_program()
    in_maps = prep_inputs(**inputs)
    res = run_bass_kernel_spmd(nc, in_maps, core_ids=list(range(N_CORES)))
    return gather_output(res.results)


# revision 17
# speedup vs baseline: 1.1644x; 1.1644x over previous
"""Trainium2 Bass kernel: transformer block with dilated (parity-strided,
banded, causal) attention.

v2 design notes (vs the earlier revision):
  * Both LayerNorms are folded into the surrounding GEMMs WITHOUT the
    mean-subtraction pass: each projection PSUM chain gets one extra rank-1
    accumulation matmul  out += mu ⊗ (-rowsum(W))  so the GEMMs consume the
    plain bf16 cast of x and never wait on the LN statistics (the rstd scale
    lands in the epilogue, the mu term is the chain tail).  This removes the
    stats->sub->GEMM serial wall that left the PE idle (and HAM-throttled to
    1.2 GHz) for ~30us per LN.
  * The dilated/causal mask is applied inside the score accumulation as an
    additive -49152 matmul (identity lhsT x host-built additive mask), so the
    gpsimd multiply (24 x 1.5us, plus a cross-engine hop in the critical
    chain) is gone; exp() of a masked score underflows to exactly 0.
  * rstd = exp(-0.5*ln(var+eps)) so the only ACT table sets used are
    natural_log_exp_and_others (stats + attention exp) and gelu_and_others
    (FFN); both are prefetched by dummy activations to hide the ~2.7us
    table-set DMA.
  * LN statistics use one PSUM bank with four concurrent M=1 column-tiled
    accumulation chains (sum/sumsq x two token chunks at partitions
    0/32/64/96).
  * Softmax denominators are collected per head into r_all and broadcast
    across partitions with one 2-row indicator matmul per feature tile
    (6 matmuls instead of 12).
"""

import numpy as np
import ml_dtypes

import concourse.bass as bass
import concourse.bacc as bacc
import concourse.mybir as mybir
import concourse.tile as tile
from concourse.bass_utils import run_bass_kernel_spmd

BF16NP = ml_dtypes.bfloat16
F32 = mybir.dt.float32
BF16 = mybir.dt.bfloat16
AF = mybir.ActivationFunctionType
OP = mybir.AluOpType

P = 128
B, L, E = 2, 2048, 768
ET = E // P            # 6 tiles over E
H, D = 12, 64
MLP = 4 * E            # 3072
MT = MLP // P          # 24
OWN = 512              # tokens owned per core
HALO = 256             # preceding-context tokens
SLAB = OWN + HALO      # 768
EPS = 1e-5
N_CORES = 8
MASKNEG = -49152.0     # exact in bf16; exp((s+MASKNEG)/8) == 0


def _fold2(apv):
    """[.., T] -> [.., 2, T//2] parity view of a stride-1 token axis."""
    return apv.rearrange("... (t two) -> ... two t", two=2)


def build_program():
    nc = bacc.Bacc("TRN2", target_bir_lowering=False, debug=False)

    xT = nc.dram_tensor("xT", [E, SLAB], F32, kind="ExternalInput").ap()
    qkv_wT = nc.dram_tensor("qkv_wT", [E, 3 * E], BF16, kind="ExternalInput").ap()
    out_wT = nc.dram_tensor("out_wT", [E, E], BF16, kind="ExternalInput").ap()
    ffn_w1T = nc.dram_tensor("ffn_w1T", [E, MLP], BF16, kind="ExternalInput").ap()
    ffn_w2T = nc.dram_tensor("ffn_w2T", [MLP, E], BF16, kind="ExternalInput").ap()
    qkv_b = nc.dram_tensor("qkv_b", [3 * E], F32, kind="ExternalInput").ap()
    out_b = nc.dram_tensor("out_b", [E], F32, kind="ExternalInput").ap()
    ffn_b1 = nc.dram_tensor("ffn_b1", [MLP], F32, kind="ExternalInput").ap()
    ffn_b2 = nc.dram_tensor("ffn_b2", [E], F32, kind="ExternalInput").ap()
    maskT = nc.dram_tensor("maskT", [2, 2, P, P], BF16, kind="ExternalInput").ap()
    identT = nc.dram_tensor("identT", [P, P], BF16, kind="ExternalInput").ap()
    ind2T = nc.dram_tensor("ind2T", [65, P], F32, kind="ExternalInput").ap()
    negsT = nc.dram_tensor("negsT", [1, 3 * E], BF16, kind="ExternalInput").ap()
    negs1T = nc.dram_tensor("negs1T", [1, MLP], BF16, kind="ExternalInput").ap()
    yT = nc.dram_tensor("yT", [E, OWN], F32, kind="ExternalOutput").ap()

    with tile.TileContext(nc) as tc:
        _emit(tc, xT, qkv_wT, out_wT, ffn_w1T, ffn_w2T,
              qkv_b, out_b, ffn_b1, ffn_b2, maskT, identT, ind2T,
              negsT, negs1T, yT)
    nc.compile()
    return nc


def _emit(tc, xT, qkv_wT, out_wT, ffn_w1T, ffn_w2T,
          qkv_b, out_b, ffn_b1, ffn_b2, maskT, identT, ind2T,
          negsT, negs1T, yT):
    from contextlib import ExitStack
    ctx = ExitStack()
    nc = tc.nc

    sing = ctx.enter_context(tc.tile_pool(name="sing", bufs=1))
    xw2_pool = ctx.enter_context(tc.tile_pool(name="xw2", bufs=1))
    wq_pool = ctx.enter_context(tc.tile_pool(name="wq", bufs=3))
    wv_pool = ctx.enter_context(tc.tile_pool(name="wv", bufs=1))
    w1_pool = ctx.enter_context(tc.tile_pool(name="w1", bufs=3))
    sq_pool = ctx.enter_context(tc.tile_pool(name="sq", bufs=2))
    ex_pool = ctx.enter_context(tc.tile_pool(name="ex", bufs=3))
    row_pool = ctx.enter_context(tc.tile_pool(name="rows", bufs=3))
    ow_pool = ctx.enter_context(tc.tile_pool(name="owp", bufs=2))
    ft_pool = ctx.enter_context(tc.tile_pool(name="ftmp", bufs=3))
    den_pool = ctx.enter_context(tc.tile_pool(name="den", bufs=3))

    # PSUM: 8 banks total = g:2 + bc:1 + st:1 + sc:2 + pv:2
    ps_main = ctx.enter_context(tc.tile_pool(name="psg", bufs=3, space="PSUM"))
    ps_st = ctx.enter_context(tc.tile_pool(name="psst", bufs=1, space="PSUM"))
    ps_attn = ctx.enter_context(tc.tile_pool(name="pssc", bufs=2, space="PSUM"))
    ps_pv = ctx.enter_context(tc.tile_pool(name="pspv", bufs=2, space="PSUM"))

    # ---------------- phase 0: input DMAs + constants ----------------
    x_sb = xw2_pool.tile([P, ET, SLAB], F32, tag="xw2", name="x_sb")
    xT_v = xT.rearrange("(o p) t -> p o t", p=P)
    for et in range(ET):
        nc.sync.dma_start(out=x_sb[:, et, :], in_=xT_v[:, et, :])

    qkvb_sb = sing.tile([P, 18], F32, tag="qkvb")
    nc.sync.dma_start(out=qkvb_sb, in_=qkv_b.rearrange("(o p) -> p o", p=P))
    outb_sb = sing.tile([P, ET], F32, tag="outb")
    nc.sync.dma_start(out=outb_sb, in_=out_b.rearrange("(o p) -> p o", p=P))
    b1_sb = sing.tile([P, MT], F32, tag="b1")
    nc.sync.dma_start(out=b1_sb, in_=ffn_b1.rearrange("(o p) -> p o", p=P))
    b2_sb = sing.tile([P, ET], F32, tag="b2")
    nc.sync.dma_start(out=b2_sb, in_=ffn_b2.rearrange("(o p) -> p o", p=P))

    ident_sb = sing.tile([P, P], BF16, tag="ident")
    nc.sync.dma_start(out=ident_sb, in_=identT)
    ind2_sb = sing.tile([65, P], F32, tag="ind2")
    nc.sync.dma_start(out=ind2_sb, in_=ind2T)
    negs_sb = sing.tile([1, 3 * ET, P], BF16, tag="negs")
    nc.sync.dma_start(out=negs_sb.rearrange("a b c -> a (b c)"), in_=negsT)
    negs1_sb = sing.tile([1, MT, P], BF16, tag="negs1")
    nc.sync.dma_start(out=negs1_sb.rearrange("a b c -> a (b c)"), in_=negs1T)

    # additive masks replicated over the head-pair dim: [key, qb, h2, kb, q]
    masks_sb = sing.tile([P, 2, 2, 2, P], BF16, tag="masks")
    for qb in range(2):
        for hrep in range(2):
            for kb in range(2):
                nc.sync.dma_start(out=masks_sb[:, qb, hrep, kb, :],
                                  in_=maskT[qb, kb])

    ones_pf = sing.tile([P, 1], BF16, tag="ones_pf")
    nc.vector.memset(ones_pf, 1.0)
    ones_row = sing.tile([1, P], BF16, tag="ones_row")
    nc.vector.memset(ones_row, 1.0)
    eps_sb = sing.tile([1, 1], F32, tag="eps")
    nc.vector.memset(eps_sb, EPS)
    scr_sb = sing.tile([1, 2], F32, tag="scr")

    # preload the natural_log_exp table set while input DMAs stream
    nc.scalar.activation(scr_sb[:, 0:1], eps_sb, AF.Exp)

    # dummy matmuls HAM-warm the PE clock while the input DMAs stream
    warm_src = sing.tile([P, 256], BF16, tag="warm_src")
    nc.gpsimd.memset(warm_src, 0.0)
    const_bf = nc.const_aps.aps[(mybir.dt.bfloat16, 1.0)]
    wps = ps_main.tile([P, 512], F32, tag="g", name="warm_ps")
    for wi in range(10):
        nc.tensor.matmul(wps[0:1, 0:256], const_bf, warm_src,
                         start=True, stop=True)

    # ---------------- folded layernorm statistics ----------------
    # Four concurrent M=1 accumulation chains into one PSUM bank, column
    # groups 0/32/64/96: sum(x) chunk0, sum(x^2) chunk0, sum(x) chunk1,
    # sum(x^2) chunk1.  Chunks: [(0,512)] for 512 tokens, +[ (512,256) ]
    # for the 768-token slab.
    def emit_ln_stats_mm(xbf, src, ntok, name):
        chunks = [(0, 512)] + ([(512, ntok - 512)] if ntok > 512 else [])
        st = ps_st.tile([P, 512], F32, tag="st", name=name)
        for et in range(ET):
            nc.vector.tensor_copy(out=xbf[:, et, :ntok], in_=src[:, et, :ntok])
            xsq = sq_pool.tile([P, ntok], BF16, tag="sq")
            nc.scalar.activation(xsq, src[:, et, :ntok], AF.Square)
            for ci, (c0, cl) in enumerate(chunks):
                r0 = 64 * ci
                nc.tensor.matmul(st[r0:r0 + 1, :cl], ones_pf,
                                 xbf[:, et, c0:c0 + cl],
                                 start=(et == 0), stop=(et == ET - 1),
                                 tile_position=(0, r0))
                nc.tensor.matmul(st[r0 + 32:r0 + 33, :cl], ones_pf,
                                 xsq[:, c0:c0 + cl],
                                 start=(et == 0), stop=(et == ET - 1),
                                 tile_position=(0, r0 + 32))
        return st, chunks

    def emit_ln_epilogue(st, chunks, ninv, mu_row, arep_sb):
        """mu_row [1,ntok] bf16; arep_sb [P,ntok] bf16 (rstd broadcast)."""
        for ci, (c0, cl) in enumerate(chunks):
            r0 = 64 * ci
            nc.scalar.activation(mu_row[:, c0:c0 + cl], st[r0:r0 + 1, :cl],
                                 AF.Copy, scale=ninv)
            musq = row_pool.tile([1, 512], F32, tag="row")
            nc.scalar.activation(musq[:, :cl], st[r0:r0 + 1, :cl], AF.Square,
                                 scale=ninv)
            var = row_pool.tile([1, 512], F32, tag="row")
            nc.vector.scalar_tensor_tensor(
                out=var[:, :cl], in0=st[r0 + 32:r0 + 33, :cl], scalar=ninv,
                in1=musq[:, :cl], op0=OP.mult, op1=OP.subtract)
            lnv = row_pool.tile([1, 512], F32, tag="row")
            nc.scalar.activation(lnv[:, :cl], var[:, :cl], AF.Ln, bias=eps_sb)
            af = row_pool.tile([1, 512], F32, tag="row")
            nc.scalar.activation(af[:, :cl], lnv[:, :cl], AF.Exp, scale=-0.5)
            a = row_pool.tile([1, 512], BF16, tag="rowb")
            nc.vector.tensor_copy(out=a[:, :cl], in_=af[:, :cl])
            arep = ps_main.tile([P, 512], F32, tag="g", name="arep_ps")
            nc.tensor.matmul(arep[:, :cl], ones_row, a[:, :cl],
                             start=True, stop=True)
            nc.vector.tensor_copy(out=arep_sb[:, c0:c0 + cl],
                                  in_=arep[:, :cl])
        return a  # last rstd row (bf16) for table-prefetch ordering

    # ---------------- phase 1: LN1 stats ----------------
    x1_bf = sing.tile([P, ET, SLAB], BF16, tag="x1_bf")
    mu1 = sing.tile([1, SLAB], BF16, tag="mu1")
    a1rep = sing.tile([P, SLAB], BF16, tag="a1rep")
    st1, chunks1 = emit_ln_stats_mm(x1_bf, x_sb, SLAB, "st1")
    emit_ln_epilogue(st1, chunks1, 1.0 / E, mu1, a1rep)

    # rstd as per-partition columns for the V epilogue (tokens on partitions)
    acol_ps = ps_main.tile([P, 512], F32, tag="g", name="acol_ps")
    one1 = ones_row[:, 0:1]
    for par in range(2):
        for kb in range(3):
            nc.tensor.matmul(acol_ps[:, par * 3 + kb:par * 3 + kb + 1],
                             _fold2(a1rep[0:1, :])[:, par, kb * P:(kb + 1) * P],
                             one1, start=(par == 0 and kb == 0),
                             stop=(par == 1 and kb == 2))
    acol_sb = sing.tile([P, 2, 3], F32, tag="acol")
    nc.vector.tensor_copy(out=acol_sb.rearrange("p a b -> p (a b)"),
                          in_=acol_ps[:, 0:6])

    # ---------------- phase 2: QKV projections (LN folded in) ----------------
    wq_view = qkv_wT.rearrange("(o p) f -> p o f", p=P)

    k_sb = sing.tile([P, ET, SLAB], BF16, tag="k_sb")
    for ft in range(ET):
        wt = wq_pool.tile([P, ET, P], BF16, tag="wq")
        nc.sync.dma_start(out=wt, in_=wq_view[:, :, E + ft * P:E + (ft + 1) * P])
        for c0, cl in [(0, 512), (512, 256)]:
            ps = ps_main.tile([P, 512], F32, tag="g")
            for et in range(ET):
                nc.tensor.matmul(ps[:, :cl], wt[:, et, :],
                                 x1_bf[:, et, c0:c0 + cl],
                                 start=(et == 0), stop=False)
            nc.tensor.matmul(ps[:, :cl], negs_sb[:, ET + ft, :],
                             mu1[:, c0:c0 + cl], start=False, stop=True)
            t = ft_pool.tile([P, 512], BF16, tag="ftb")
            nc.vector.tensor_mul(t[:, :cl], ps[:, :cl], a1rep[:, c0:c0 + cl])
            nc.scalar.activation(k_sb[:, ft, c0:c0 + cl], t[:, :cl],
                                 AF.Identity, bias=qkvb_sb[:, 6 + ft:7 + ft])

    q_sb = sing.tile([P, ET, OWN], BF16, tag="q_sb")
    for ft in range(ET):
        wt = wq_pool.tile([P, ET, P], BF16, tag="wq")
        nc.sync.dma_start(out=wt, in_=wq_view[:, :, ft * P:(ft + 1) * P])
        ps = ps_main.tile([P, 512], F32, tag="g")
        for et in range(ET):
            nc.tensor.matmul(ps, wt[:, et, :], x1_bf[:, et, HALO:SLAB],
                             start=(et == 0), stop=False)
        nc.tensor.matmul(ps, negs_sb[:, ft, :], mu1[:, HALO:SLAB],
                         start=False, stop=True)
        t = ft_pool.tile([P, 512], BF16, tag="ftb")
        nc.vector.tensor_mul(t, ps, a1rep[:, HALO:SLAB])
        nc.scalar.activation(q_sb[:, ft, :], t, AF.Identity,
                             bias=qkvb_sb[:, ft:ft + 1])

    # V in [token, feature] orientation; LN fold: rank-1 mu term via folded
    # mu as lhsT, rstd applies per-partition via ACT scale.  V bias is folded
    # into out_b on the host.
    v_sb = sing.tile([P, 2, 3, H, D + 1], BF16, tag="v_sb")
    nc.vector.memset(v_sb[:, :, :, :, D:D + 1], 1.0)
    for vc0, vcl in [(0, 512), (512, 256)]:
        wt = wv_pool.tile([P, ET, vcl], BF16, tag=f"wv{vcl}", name=f"wtv{vcl}")
        nc.sync.dma_start(out=wt,
                          in_=wq_view[:, :, 2 * E + vc0:2 * E + vc0 + vcl])
        for kb in range(3):
            for par in range(2):
                ps = ps_main.tile([P, 512], F32, tag="g")
                for et in range(ET):
                    hblk = _fold2(x1_bf[:, et, :])[:, par, kb * P:(kb + 1) * P]
                    nc.tensor.matmul(ps[:, :vcl], hblk, wt[:, et, :vcl],
                                     start=(et == 0), stop=False)
                nsv = negs_sb[:, 2 * ET + vc0 // P:2 * ET + (vc0 + vcl) // P,
                              :].rearrange("a b c -> a (b c)")
                nc.tensor.matmul(ps[:, :vcl],
                                 _fold2(mu1)[:, par, kb * P:(kb + 1) * P],
                                 nsv, start=False, stop=True)
                nc.scalar.activation(
                    v_sb[:, par, kb, vc0 // D:(vc0 + vcl) // D, 0:D],
                    ps[:, :vcl].rearrange("p (h d) -> p h d", d=D), AF.Copy,
                    scale=acol_sb[:, par, kb:kb + 1])

    outw_v = out_wT.rearrange("(o p) e -> p o e", p=P)

    # ---------------- phase 3: dilated attention ----------------
    o_sb = sing.tile([P, ET, OWN], BF16, tag="o_sb")
    r_all = sing.tile([65, ET, 2, 256], F32, tag="r_all")
    pairs = [(0, 2), (1, 3), (4, 6), (5, 7), (8, 10), (9, 11)]
    for pi, (h0, h1) in enumerate(pairs):
        kt = h0 // 2
        ro = D * (h0 % 2)
        slot = h0 % 2
        for par in range(2):
            for qb in range(2):
                sc = ps_attn.tile([P, 2, 2, P], F32, tag="sc")
                nc.tensor.matmul(sc.rearrange("p a b c -> p (a b c)"),
                                 ident_sb, masks_sb[:, qb].rearrange(
                                     "p a b c -> p (a b c)"),
                                 start=True, stop=False)
                for hi, h in enumerate((h0, h1)):
                    ktt = h // 2
                    qv = _fold2(q_sb[ro:ro + D, ktt, :])[:, par,
                                                         qb * P:(qb + 1) * P]
                    kv = _fold2(k_sb[ro:ro + D, ktt, :])
                    for kbi, kb in enumerate((qb, qb + 1)):
                        nc.tensor.matmul(
                            sc[:, hi, kbi, :],
                            kv[:, par, kb * P:(kb + 1) * P], qv,
                            start=False,
                            stop=(hi == 1 and kbi == 1))
                ex = ex_pool.tile([P, 2, 2, P], BF16, tag="ex")
                nc.scalar.activation(ex, sc, AF.Exp, scale=1.0 / np.sqrt(D))
                pv = ps_pv.tile([D + 1, 2, P], F32, tag="pv")
                for hi, h in enumerate((h0, h1)):
                    for kbi, kb in enumerate((qb, qb + 1)):
                        nc.tensor.matmul(
                            pv[:, hi, :], v_sb[:, par, kb, h, :],
                            ex[:, hi, kbi, :],
                            start=(hi == 0 and kbi == 0),
                            stop=(hi == 1 and kbi == 1))
                den = den_pool.tile([1, 2, P], F32, tag="den")
                nc.vector.tensor_copy(out=den, in_=pv[D:D + 1, :, :])
                if slot == 0:
                    nc.vector.reciprocal_approx_fast(
                        out=r_all[0:1, kt:kt + 2, par, qb * P:(qb + 1) * P],
                        in_=den)
                else:
                    den2 = den_pool.tile([1, 2, P], F32, tag="den2")
                    nc.vector.reciprocal_approx_fast(out=den2, in_=den)
                    nc.vector.tensor_copy(
                        out=r_all[64:65, kt:kt + 2, par, qb * P:(qb + 1) * P],
                        in_=den2)
                dst = _fold2(o_sb[ro:ro + D, kt:kt + 2, :])[:, :, par,
                                                            qb * P:(qb + 1) * P]
                nc.vector.tensor_copy(out=dst, in_=pv[0:D])
        if pi % 2 == 1:
            # both row-halves of feature tiles kt, kt+1 are complete
            for tt in (kt, kt + 1):
                rrep = ps_main.tile([P, 512], F32, tag="g", name="rrep_ps")
                rrv = rrep.rearrange("m (q p) -> m q p", p=2)
                for s in range(2):
                    nc.tensor.matmul(
                        rrv, ind2_sb[64 * s:64 * s + 1, :],
                        r_all[64 * s:64 * s + 1, tt].rearrange(
                            "a p q -> a q p"),
                        start=(s == 0), stop=(s == 1))
                nc.vector.tensor_mul(o_sb[:, tt, :], o_sb[:, tt, :], rrep)

    # ---------------- phase 4: out-proj + residual ----------------
    y1_sb = sing.tile([P, ET, OWN], F32, tag="y1_sb")
    for et in range(ET):
        owt = ow_pool.tile([P, ET, P], BF16, tag="ow")
        nc.sync.dma_start(out=owt, in_=outw_v[:, :, et * P:(et + 1) * P])
        ps = ps_main.tile([P, 512], F32, tag="g")
        for ftl in range(ET):
            nc.tensor.matmul(ps, owt[:, ftl, :],
                             o_sb[:, ftl, :],
                             start=(ftl == 0), stop=(ftl == ET - 1))
        t = ft_pool.tile([P, 512], F32, tag="ft")
        nc.scalar.activation(t, ps, AF.Identity, bias=outb_sb[:, et:et + 1])
        nc.vector.tensor_add(y1_sb[:, et, :], t, x_sb[:, et, HALO:SLAB])

    # ---------------- phase 5: LN2 stats ----------------
    y1_bf = sing.tile([P, ET, OWN], BF16, tag="y1_bf")
    mu2 = sing.tile([1, OWN], BF16, tag="mu2")
    a2rep = sing.tile([P, OWN], BF16, tag="a2rep")
    st2, chunks2 = emit_ln_stats_mm(y1_bf, y1_sb, OWN, "st2")
    a2row = emit_ln_epilogue(st2, chunks2, 1.0 / E, mu2, a2rep)

    # prefetch the gelu table set now that the last exp (rstd2) is emitted
    nc.scalar.activation(scr_sb[:, 1:2], a2row[:, 0:1], AF.Gelu)

    # ---------------- phase 6: FFN1 + GELU (LN folded in) ----------------
    w2_sb = xw2_pool.tile([P, MT, E], BF16, tag="xw2", name="w2_sb")
    w2_v = ffn_w2T.rearrange("(o p) e -> p o e", p=P)
    for ktl in range(MT):
        nc.sync.dma_start(out=w2_sb[:, ktl, :], in_=w2_v[:, ktl, :])

    w1_view = ffn_w1T.rearrange("(o p) f -> p o f", p=P)
    ffnh = sing.tile([P, MT, OWN], BF16, tag="ffnh")
    for mt in range(MT):
        wt = w1_pool.tile([P, ET, P], BF16, tag="w1")
        nc.sync.dma_start(out=wt, in_=w1_view[:, :, mt * P:(mt + 1) * P])
        ps = ps_main.tile([P, 512], F32, tag="g")
        for et in range(ET):
            nc.tensor.matmul(ps, wt[:, et, :], y1_bf[:, et, :],
                             start=(et == 0), stop=False)
        nc.tensor.matmul(ps, negs1_sb[:, mt, :], mu2,
                         start=False, stop=True)
        t = ft_pool.tile([P, 512], BF16, tag="ftb")
        nc.vector.tensor_mul(t, ps, a2rep)
        nc.scalar.activation(ffnh[:, mt, :], t, AF.Gelu,
                             bias=b1_sb[:, mt:mt + 1])

    # ---------------- phase 7: FFN2 + residual + store ----------------
    yT_view = yT.rearrange("(o p) t -> p o t", p=P)
    for et in range(ET):
        ps = ps_main.tile([P, 512], F32, tag="g")
        for ktl in range(MT):
            nc.tensor.matmul(ps, w2_sb[:, ktl, et * P:(et + 1) * P],
                             ffnh[:, ktl, :],
                             start=(ktl == 0), stop=(ktl == MT - 1))
        t = ft_pool.tile([P, 512], F32, tag="ft")
        nc.scalar.activation(t, ps, AF.Identity, bias=b2_sb[:, et:et + 1])
        nc.vector.tensor_add(y1_sb[:, et, :], t, y1_sb[:, et, :])
        nc.sync.dma_start(out=yT_view[:, et, :], in_=y1_sb[:, et, :])

    ctx.close()


# ======================= host side =======================

def prep_inputs(x, ln1_w, ln1_b, qkv_w, qkv_b, out_w, out_b,
                ln2_w, ln2_b, ffn_w1, ffn_b1, ffn_w2, ffn_b2):
    """Shard/fold/cast the full inputs into 8 per-core input maps."""
    x = np.asarray(x, np.float32)
    f8 = lambda v: np.asarray(v, np.float64)

    qkv_wp = f8(qkv_w) * f8(ln1_w)[None, :]
    qkv_wT = qkv_wp.T.astype(BF16NP).copy()
    qkv_b_eff = (f8(qkv_b) + f8(qkv_w) @ f8(ln1_b)).astype(np.float32)
    out_wT = f8(out_w).T.astype(BF16NP).copy()
    out_b_eff = (f8(out_b) + f8(out_w) @ f8(qkv_b)[2 * E:]).astype(np.float32)
    ffn_w1p = f8(ffn_w1) * f8(ln2_w)[None, :]
    ffn_w1T = ffn_w1p.T.astype(BF16NP).copy()
    ffn_b1_eff = (f8(ffn_b1) + f8(ffn_w1) @ f8(ln2_b)).astype(np.float32)
    ffn_w2T = f8(ffn_w2).T.astype(BF16NP).copy()
    ffn_b2_f = np.asarray(ffn_b2, np.float32)

    # row-sums of the (ln-folded) weights, negated, for the rank-1 mu fold
    negs = (-qkv_wp.sum(axis=1)).astype(BF16NP).reshape(1, 3 * E)
    negs1 = (-ffn_w1p.sum(axis=1)).astype(BF16NP).reshape(1, MLP)

    ident = np.eye(P, dtype=BF16NP)
    ind2 = np.zeros((65, P), dtype=np.float32)
    ind2[0, 0:D] = 1.0
    ind2[64, D:P] = 1.0

    cidx = np.arange(P)[:, None]   # key (folded, within block)
    ridx = np.arange(P)[None, :]   # query (folded, within block)
    m_prev = np.where(cidx >= ridx, 0.0, MASKNEG).astype(BF16NP)
    m_diag = np.where(cidx <= ridx, 0.0, MASKNEG).astype(BF16NP)
    m_none = np.full((P, P), MASKNEG, BF16NP)

    in_maps = []
    for c in range(N_CORES):
        b, ch = divmod(c, 4)
        lo = OWN * ch - HALO
        if ch == 0:
            slab = np.concatenate(
                [np.zeros((HALO, E), np.float32), x[b, 0:OWN]], axis=0)
        else:
            slab = x[b, lo:lo + SLAB]
        xTc = np.ascontiguousarray(slab.T)

        mask = np.stack([
            np.stack([m_none if ch == 0 else m_prev, m_diag]),  # qb = 0
            np.stack([m_prev, m_diag]),                         # qb = 1
        ]).astype(BF16NP)

        in_maps.append({
            "xT": xTc,
            "qkv_wT": qkv_wT, "out_wT": out_wT,
            "ffn_w1T": ffn_w1T, "ffn_w2T": ffn_w2T,
            "qkv_b": qkv_b_eff, "out_b": out_b_eff,
            "ffn_b1": ffn_b1_eff, "ffn_b2": ffn_b2_f,
            "maskT": np.ascontiguousarray(mask),
            "identT": ident, "ind2T": ind2,
            "negsT": negs, "negs1T": negs1,
        })
    return in_maps


def gather_output(results):
    y = np.empty((B, L, E), np.float32)
    for c in range(N_CORES):
        b, ch = divmod(c, 4)
        y[b, OWN * ch:OWN * (ch + 1)] = results[c]["yT"].T
    return y


_NC_CACHE = None


def _get_program():
    global _NC_CACHE
    if _NC_CACHE is None:
        _NC_CACHE = build_program()
    return _NC_CACHE


def kernel(**inputs):
    nc = _get_program()
    in_maps = prep_inputs(**inputs)
    res = run_bass_kernel_spmd(nc, in_maps, core_ids=list(range(N_CORES)))
    return gather_output(res.results)
